# revision 53
# baseline (speedup 1.0000x reference)
"""Causal full-d_model attention (B=4, T=2048, C=1024) on 8 Trainium2 cores.

Sharding: core = 2*b + p handles batch b and two 512-row sequence blocks,
p=0 -> blocks {0, 3}, p=1 -> blocks {1, 2} (pairing balances causal work).

K is hybrid-redundant: every core projects K for global blocks {0, 1}
(the keys every query block needs) straight into SBUF, plus its OWN hi
block (3 on p=0, 2 on p=1), which is staged to DRAM and exchanged with
the pair partner by a 1MB AllGather ([[0,1],[2,3],...] - the valid
LNC1_4x2 shape).  kag_out is rank-indexed but rank r's hi block is the
same global block on both cores (rank 0 always staged block 3, rank 1
block 2), so the readback into global kT slots is SPMD-uniform.  The
redundant block-{0,1} projection costs ~13.7us of extra PE work but
takes the K collective off the scores critical path entirely: slot-A
scores need only locally-drained kT, and slot-B consumes the exchanged
blocks ~70us after the AllGather lands.

V stays token-split (each core projects its own 1024-token half; a 2MB
AllGather + readback assembles the full v), because PV runs last and
gives that collective ~100us of compute to hide under.

All matmul operands are bf16 (same PE rate as f32r, but FWL halves
LDWEIGHTS and every DMA byte count halves).  Accumulation is fp32 in
PSUM; softmax denominator / reciprocal / biases stay fp32.

On-device layout is transposed ([feature, token]) so every matmul
contracts along the partition axis:
    kT/qT = W.T @ xT            (projection)
    v     = xT.T @ Wv           (token-partition layout)
    scoresT[j, i] = kT_j.T @ qT (j on partitions)
    attnT[c, i]  += v_j.T @ probsT
    outT          = Wo_slice.T @ attnT
Softmax is unnormalized (scores ~ N(0,1), exp is safe); the denominator
comes from an M=1 ones-column matmul over masked exp tiles and is applied
at the PSUM->SBUF copy via a DRAM-broadcast reciprocal row.  Causal masks
arrive as per-core input data, so all 8 cores run one SPMD program.

DMA plan: every transfer is per-partition contiguous (the 256B-run
panel loads of earlier revisions ran at descriptor-dominated rates).
Two HWDGE rings run in parallel: sync carries bias + Wk then goes idle
so the staging chunks and kT readback fire the moment they are ready;
scalar carries x / masks / Wv / Wq / Wo (need-ordered) and the slot-B
output.  The v readback is emitted after slot-A's reciprocal DMAs so
nothing early queues behind a trigger that waits on the V AllGather.
kT_sb's hi slot doubles as the K staging buffer (the readback
overwrites it after the AllGather consumed it), saving SBUF.
"""

import math

import numpy as np

P = 128          # SBUF partitions
B_, T_, C_ = 4, 2048, 1024
RG = [[0, 1], [2, 3], [4, 5], [6, 7]]   # intra-pair replica groups


def _emit(nc, tc, aps, T, C):
    import concourse.bass as bass
    from concourse import mybir
    from concourse.tile_rust import add_dep_helper
    from contextlib import ExitStack

    AFT = mybir.ActivationFunctionType
    f32 = mybir.dt.float32
    bf16 = mybir.dt.bfloat16

    NT = C // P            # feature tiles (8)
    BLK = T // 4           # sequence block = i-slot width FB (512)
    TL = 2 * BLK           # local query tokens per core
    TH = T // 2            # V tokens projected per core
    FB = BLK               # matmul moving free dim
    NCHL = TH // BLK       # local V chunks (2)
    NBL = BLK // P         # 128-key tiles per 512 block (4)
    NS = 4                 # query slots per core, 256 rows each
    QW = TL // NS          # slot width (256)
    MW = 3 * P + QW        # shared mask-master width (640)
    SC = 1.0 / math.sqrt(C)

    (xk, xv, xq, Wk, Wq, Wv, Wo, bias_t, masks_t, rec_dram, outT) = aps

    with ExitStack() as ctx:
        singles = ctx.enter_context(tc.tile_pool(name="singles", bufs=1))
        kpool = ctx.enter_context(tc.tile_pool(name="kpool", bufs=1))
        qpool = ctx.enter_context(tc.tile_pool(name="qpool", bufs=1))
        vpool = ctx.enter_context(tc.tile_pool(name="vpool", bufs=1))
        psp = ctx.enter_context(tc.tile_pool(name="psp", bufs=8, space="PSUM"))
        dramp = ctx.enter_context(tc.tile_pool(name="dramp", bufs=1,
                                               space="DRAM"))

        # bias3 packs bq/bk/bo + an fp32 ones column; m_all is the single
        # sliding mask master: with slot key-counts 4(s+1) the causal
        # window offset reduces to the per-core constant 384-256p, so one
        # [P, 640] master serves every (slot, j-tile) pair on every core.
        bias3 = singles.tile([P, 4, NT], f32, name="bias3")
        m_all = singles.tile([P, MW], bf16, name="m_all")
        bq_sb, bk_sb, bo_sb = bias3[:, 0], bias3[:, 1], bias3[:, 2]
        ones_f32 = bias3[:, 3, 0:1]

        # kT_sb is block-major: kT_sb[:, b, ci, t'] = k[b*BLK + t', ci*P+p]
        kT_sb = kpool.tile([P, 4, NT, BLK], bf16, name="kT_sb")
        qT_sb = qpool.tile([P, NT, TL], bf16, name="qT_sb")
        v_sb = vpool.tile([P, T // P, C], bf16, name="v_sb")

        # DRAM bounce buffers (per-partition contiguous pieces)
        kag_in = dramp.tile([P, NT, BLK], bf16, name="kag_in")
        kag_out = dramp.tile([2, P, NT, BLK], bf16, name="kag_out")
        vag_in = dramp.tile([P, NCHL, NBL, C], bf16, name="vag_in")
        vag_out = dramp.tile([2, P, NCHL, NBL, C], bf16, name="vag_out")

        # ------- phase 1: K (hi + shared), V-half (+AllGathers), Q -------
        with ExitStack() as p1:
            wpool = p1.enter_context(tc.tile_pool(name="wpool", bufs=1))
            xkpool = p1.enter_context(tc.tile_pool(name="xkpool", bufs=1))
            xvpool = p1.enter_context(tc.tile_pool(name="xvpool", bufs=1))
            xqpool = p1.enter_context(tc.tile_pool(name="xqpool", bufs=1))
            vstp = p1.enter_context(tc.tile_pool(name="vstp", bufs=2))

            wk_sb = wpool.tile([P, NT, NT, P], bf16, name="wk_sb")
            wq_sb = wpool.tile([P, NT, NT, P], bf16, name="wq_sb")
            wv_sb = wpool.tile([P, NT, C], bf16, name="wv_sb")
            xk_sb = xkpool.tile([P, 3, NT, BLK], bf16, name="xk_sb")
            xv_sb = xvpool.tile([P, NCHL, NT, BLK], bf16, name="xv_sb")
            xq_sb = xqpool.tile([P, 2, NT, FB], bf16, name="xq_sb")

            # sync ring: Wk in 512KB co-quarters so the K matmuls start on
            # quarter 1 and consume the rest progressively, then Wv; the
            # ring then drains so staging/readbacks fire when ready
            for q in range(4):
                nc.sync.dma_start(out=wk_sb[:, 2 * q:2 * q + 2],
                                  in_=Wk[:, 2 * q:2 * q + 2])
            nc.sync.dma_start(out=wv_sb[:, :NT // 2], in_=Wv[:, :NT // 2])
            # scalar ring: bias first (the K-hi drains need it early),
            # then x chunks + late weights, need-ordered
            nc.scalar.dma_start(out=bias3, in_=bias_t)
            nc.scalar.dma_start(out=xk_sb[:, 2], in_=xk[:, 2])
            nc.scalar.dma_start(out=xk_sb[:, 0], in_=xk[:, 0])
            nc.scalar.dma_start(out=xv_sb[:, 0], in_=xv[:, 0])
            nc.scalar.dma_start(out=xv_sb[:, 1], in_=xv[:, 1])
            nc.scalar.dma_start(out=xk_sb[:, 1], in_=xk[:, 1])
            nc.scalar.dma_start(out=xq_sb, in_=xq)
            nc.scalar.dma_start(out=m_all, in_=masks_t)
            nc.scalar.dma_start(out=wq_sb[:, :NT // 2], in_=Wq[:, :NT // 2])
            nc.scalar.dma_start(out=wq_sb[:, NT // 2:], in_=Wq[:, NT // 2:])

            # HAM warm-up: tiny matmuls on a memset tile (ready ~6.5us,
            # before any DMA data lands) bridge the startup window so the
            # PE clock gate hits 8/8 before the real K work starts.
            wset = wpool.tile([P, NT], f32, name="wset")
            nc.vector.memset(wset, 1.0)
            ps_w = psp.tile([P, BLK], f32, name="ps_w", tag="ps")
            for i in range(56):
                nc.tensor.matmul(
                    ps_w[0:8, 0:8],
                    wset,
                    wset,
                    start=True,
                    stop=True,
                    skip_group_check=True,
                )

            # K: kT = Wk.T @ x (+bk).  The own-hi chunk runs first and
            # drains into kT_sb slot 3 (used as staging scratch; the
            # post-AllGather readback overwrites slots 2 and 3 with the
            # global hi blocks); blocks 0/1 drain into their global slots
            # directly, after V so the V AllGather also triggers early.
            def emit_kproj(src, dst):
                for co in range(NT):
                    ps = psp.tile([P, BLK], f32, name="ps_k", tag="ps")
                    for ci in range(NT):
                        nc.tensor.matmul(
                            ps,
                            wk_sb[:, co, ci, :],
                            xk_sb[:, src, ci, :],
                            start=(ci == 0),
                            stop=(ci == NT - 1),
                        )
                    nc.scalar.activation(
                        out=kT_sb[:, dst, co, :], in_=ps, func=AFT.Identity,
                        bias=bk_sb[:, co:co + 1],
                    )

            emit_kproj(2, 3)
            nc.sync.dma_start(out=kag_in, in_=kT_sb[:, 3])
            nc.gpsimd.collective_compute(
                "AllGather",
                mybir.AluOpType.bypass,
                replica_groups=RG,
                ins=[kag_in.opt()],
                outs=[kag_out.opt()],
            )
            # second Wv half rides the sync ring BEHIND the K staging
            # chunk, so the collective chain starts ~8us earlier than when
            # a whole 2MB Wv preceded it; the V matmuls' ci 4-7 steps wait
            # ~2us for it, hidden almost entirely by the ci 0-3 steps
            nc.sync.dma_start(out=wv_sb[:, NT // 2:], in_=Wv[:, NT // 2:])
            # K block 0 before V: gives the scalar ring 14us more to land
            # xv, and slot-A's first kT arrives that much sooner
            emit_kproj(0, 0)

            # V-half: v_loc = x_half @ Wv; PSUM drains land in a per-chunk
            # staging tile that ships as one contiguous 1MB DMA
            # (bv is folded into bo_t on the host)
            for l in range(NCHL):
                vst = vstp.tile([P, NBL, C], bf16, name="vst", tag="vst")
                for jt in range(NBL):
                    for ch in range(C // 512):
                        ps = psp.tile([P, 512], f32, name="ps_v", tag="ps")
                        for ci in range(NT):
                            nc.tensor.matmul(
                                ps,
                                xv_sb[:, l, ci, jt * P:(jt + 1) * P],
                                wv_sb[:, ci, ch * 512:(ch + 1) * 512],
                                start=(ci == 0),
                                stop=(ci == NT - 1),
                            )
                        nc.vector.tensor_copy(
                            vst[:, jt, ch * 512:(ch + 1) * 512], ps
                        )
                nc.sync.dma_start(out=vag_in[:, l], in_=vst)
            nc.gpsimd.collective_compute(
                "AllGather",
                mybir.AluOpType.bypass,
                replica_groups=RG,
                ins=[vag_in.opt()],
                outs=[vag_out.opt()],
            )

            # the second shared K block, then the hi-block readback: rank
            # 0's staged chunk is always global block 3, rank 1's always
            # block 2, on every core of the pair
            emit_kproj(1, 1)
            nc.sync.dma_start(out=kT_sb[:, 2], in_=kag_out[1])
            nc.sync.dma_start(out=kT_sb[:, 3], in_=kag_out[0])

            # v readback, blocks 0/1 on the sync ring (blocks 2/3 go on
            # the scalar ring, but are emitted in phase 2 after the score
            # exps - a scalar-queue DMA trigger that waits on the V
            # AllGather would block every later ACT instruction)
            for l in range(NCHL):
                nc.sync.dma_start(
                    out=v_sb[:, l * NBL:(l + 1) * NBL, :],
                    in_=vag_out[0][:, l],
                )

            # Q: qT = Wq.T @ xq (+bq) for the two local 512-blocks
            for s in range(2):
                for co in range(NT):
                    ps = psp.tile([P, FB], f32, name="ps_q", tag="ps")
                    for ci in range(NT):
                        nc.tensor.matmul(
                            ps,
                            wq_sb[:, co, ci, :],
                            xq_sb[:, s, ci, :],
                            start=(ci == 0),
                            stop=(ci == NT - 1),
                        )
                    nc.scalar.activation(
                        out=qT_sb[:, co, s * FB:(s + 1) * FB],
                        in_=ps,
                        func=AFT.Identity,
                        bias=bq_sb[:, co:co + 1],
                    )

        # -------- phase 2: attention + output projection --------
        with ExitStack() as p2:
            wopool = p2.enter_context(tc.tile_pool(name="wopool", bufs=1))
            probsp = p2.enter_context(tc.tile_pool(name="probsp", bufs=40))
            attnp = p2.enter_context(tc.tile_pool(name="attnp", bufs=1))
            recp = p2.enter_context(tc.tile_pool(name="recp", bufs=NS))
            ostagep = p2.enter_context(tc.tile_pool(name="ostagep", bufs=2))

            # Wo loads into SBUF freed by the phase-1 pools; needed ~100us
            # after this fires
            wo_sb = wopool.tile([P, NT, NT, P], bf16, name="wo_sb")
            nc.scalar.dma_start(out=wo_sb, in_=Wo)

            def emit_scores(s):
                # slot s = the core's s-th 256-row sub-block (global
                # sub-block 2s+p); key-tile count 4(s+1) covers both
                # cores' needs, j-tiles below 4s are causally all-ones
                nj, j0m = 4 * (s + 1), 4 * s
                pjs = []
                # per-partition fp32 running sum of the masked exp tiles;
                # one M=1 fp32 matmul at the end folds the partition axis,
                # replacing nj N=256 PE instructions with DVE adds that
                # hide under the score matmuls
                acc = recp.tile([P, QW], f32, name="acc", tag="acc")
                for jt in range(nj):
                    blk, r = divmod(jt, NBL)
                    ps_s = psp.tile([P, FB], f32, name="ps_s", tag="ps")
                    for ci in range(NT):
                        nc.tensor.matmul(
                            ps_s[:, :QW],
                            kT_sb[:, blk, ci, r * P:(r + 1) * P],
                            qT_sb[:, ci, s * QW:(s + 1) * QW],
                            start=(ci == 0),
                            stop=(ci == NT - 1),
                        )
                    pj = probsp.tile([P, QW], bf16, name="pj", tag="pj")
                    nc.scalar.activation(out=pj, in_=ps_s[:, :QW],
                                         func=AFT.Exp, scale=SC)
                    if jt >= j0m:  # earlier j-tiles are all-ones everywhere
                        s0 = (3 - (jt - j0m)) * P
                        nc.vector.tensor_mul(pj, pj, m_all[:, s0:s0 + QW])
                    if jt == 0:
                        nc.vector.tensor_copy(acc, pj)
                    else:
                        nc.vector.tensor_add(acc, acc, pj)
                    pjs.append(pj)
                return pjs, acc

            def emit_den(s, acc):
                # deferred past all score passes so the PE never waits on
                # the DVE accumulation chains
                ps_den = psp.tile([1, QW], f32, name="ps_den", tag="ps")
                nc.tensor.matmul(
                    ps_den, ones_f32, acc, start=True, stop=True,
                    skip_group_check=True,
                )
                # 1/denominator: quick copy releases the PSUM bank, then the
                # slow reciprocal runs off the SBUF copy; broadcast to 128
                # partitions via a stride-0 DRAM read.
                den_sb = recp.tile([1, QW], f32, name="den_sb", tag="den_sb")
                nc.scalar.copy(den_sb, ps_den)
                rrow = recp.tile([1, QW], f32, name="rrow", tag="rrow")
                nc.vector.reciprocal(rrow, den_sb)
                rec_w = nc.sync.dma_start(out=rec_dram[s:s + 1, :], in_=rrow)
                recipB = recp.tile([P, QW], f32, name="recipB", tag="recipB")
                rec_row = rec_dram[s, :]
                rec_bcast = bass.AP(
                    tensor=rec_row.tensor,
                    offset=rec_row.offset,
                    ap=[[0, P]] + [list(d) for d in rec_row.ap],
                )
                rec_r = nc.sync.dma_start(out=recipB, in_=rec_bcast)
                add_dep_helper(rec_r.ins, rec_w.ins, reason="rec_dram RAW")
                return recipB

            def emit_pv_group(s, pjs, recipB, g0):
                # one 4-bank PSUM accumulation group of PV; the attn
                # multiplies drain the banks while later PE work runs
                nj = 4 * (s + 1)
                ps_attn = [
                    psp.tile([P, FB], f32, name="ps_attn", tag="ps")
                    for _ in range(NT // 2)
                ]
                for jt in range(nj):
                    for k, ct in enumerate(range(g0, g0 + NT // 2)):
                        nc.tensor.matmul(
                            ps_attn[k][:, :QW],
                            v_sb[:, jt, ct * P:(ct + 1) * P],
                            pjs[jt],
                            start=(jt == 0),
                            stop=(jt == nj - 1),
                            skip_group_check=True,
                        )
                for k, ct in enumerate(range(g0, g0 + NT // 2)):
                    nc.vector.tensor_mul(
                        attn_all[:, ct, s * QW:(s + 1) * QW],
                        ps_attn[k][:, :QW], recipB,
                    )

            def emit_oproj(u):
                # N=512 output matmuls sweep two adjacent 256 slots
                for co in range(NT):
                    # alternate rings so the final output pieces overlap
                    dma_q = nc.sync if co % 2 == u else nc.scalar
                    ps_o = psp.tile([P, FB], f32, name="ps_o", tag="ps")
                    for ci in range(NT):
                        nc.tensor.matmul(
                            ps_o,
                            wo_sb[:, co, ci, :],
                            attn_all[:, ci, u * FB:(u + 1) * FB],
                            start=(ci == 0),
                            stop=(ci == NT - 1),
                        )
                    os_ = ostagep.tile([P, FB], f32, name="os_", tag="os")
                    nc.scalar.activation(
                        out=os_, in_=ps_o, func=AFT.Identity,
                        bias=bo_sb[:, co:co + 1],
                    )
                    dma_q.dma_start(
                        out=outT[co * P:(co + 1) * P, u * FB:(u + 1) * FB],
                        in_=os_,
                    )

            # All score passes run before any PV: the v AllGather +
            # readback gets the whole scores span to hide under before
            # PV reads v.  O-proj u=0 sits between PV slots 2 and 3 so
            # the attn-multiply latency hides under other PE work.
            pjss, accs = [], []
            for s in range(NS):
                pjs, acc = emit_scores(s)
                pjss.append(pjs)
                accs.append(acc)

            # v readback blocks 2/3 on the scalar ring, emitted only now:
            # every ACT instruction before this point (all score exps) has
            # dispatched by the time these triggers wait on the AllGather
            for l in range(NCHL):
                nc.scalar.dma_start(
                    out=v_sb[:, (2 + l) * NBL:(3 + l) * NBL, :],
                    in_=vag_out[1][:, l],
                )

            recBs = [emit_den(s, acc) for s, acc in enumerate(accs)]
            attn_all = attnp.tile([P, NT, TL], bf16, name="attn_all",
                                  tag="attn")
            for s in range(NS):
                emit_pv_group(s, pjss[s], recBs[s], 0)
                emit_pv_group(s, pjss[s], recBs[s], NT // 2)
                if s == 2:
                    emit_oproj(0)
            emit_oproj(1)


def build_program(T=T_, C=C_, num_cores=8):
    """Build and compile the SPMD Bass program."""
    from concourse import bacc, mybir
    import concourse.tile as tile

    f32 = mybir.dt.float32
    bf16 = mybir.dt.bfloat16
    NT = C // P
    BLK = T // 4
    TL = 2 * BLK
    QW = TL // 4
    MW = 3 * P + QW

    nc = bacc.Bacc(
        "TRN2", target_bir_lowering=False, debug=False, num_devices=num_cores
    )
    xk = nc.dram_tensor("xk", [P, 3, NT, BLK], bf16, kind="ExternalInput").ap()
    xv = nc.dram_tensor("xv", [P, 2, NT, BLK], bf16, kind="ExternalInput").ap()
    xq = nc.dram_tensor("xq", [P, 2, NT, BLK], bf16, kind="ExternalInput").ap()
    Wk = nc.dram_tensor("Wk", [P, NT, NT, P], bf16, kind="ExternalInput").ap()
    Wq = nc.dram_tensor("Wq", [P, NT, NT, P], bf16, kind="ExternalInput").ap()
    Wv = nc.dram_tensor("Wv", [P, NT, C], bf16, kind="ExternalInput").ap()
    Wo = nc.dram_tensor("Wo", [P, NT, NT, P], bf16, kind="ExternalInput").ap()
    bias_t = nc.dram_tensor("bias_t", [P, 4, NT], f32,
                            kind="ExternalInput").ap()
    masks_t = nc.dram_tensor("masks_t", [P, MW], bf16,
                             kind="ExternalInput").ap()
    rec_dram = nc.dram_tensor("rec_int", [4, QW], f32).ap()
    outT = nc.dram_tensor("outT", [C, TL], f32, kind="ExternalOutput").ap()

    aps = (xk, xv, xq, Wk, Wq, Wv, Wo, bias_t, masks_t, rec_dram, outT)
    with tile.TileContext(nc) as tc:
        _emit(nc, tc, aps, T, C)
    nc.compile()
    return nc


def make_core_inputs(x, Wq, bq, Wk, bk, Wv, bv, Wo, bo, T=T_, C=C_):
    """Per-core input maps (list of 8 dicts) for the SPMD program."""
    import ml_dtypes

    f = np.float32
    bf = ml_dtypes.bfloat16
    NT = C // P
    BLK = T // 4
    QW = T // 8
    MW = 3 * P + QW

    x = np.asarray(x, f)
    Wq, Wk, Wv, Wo = (np.asarray(w, f) for w in (Wq, Wk, Wv, Wo))
    bq, bk, bv, bo = (np.asarray(b, f) for b in (bq, bk, bv, bo))

    def panels(W):  # [C, C] -> [P, co, ci, m]: W[ci*P+p, co*P+m]
        return np.ascontiguousarray(
            W.reshape(NT, P, NT, P).transpose(1, 2, 0, 3)
        ).astype(bf)

    Wk_t = panels(Wk)
    Wq_t = panels(Wq)
    Wo_t = panels(Wo)
    # [C, C] -> [P, ci, m]: Wv[ci*P+p, m]
    Wv_t = np.ascontiguousarray(
        Wv.reshape(NT, P, C).transpose(1, 0, 2)
    ).astype(bf)
    bo_eff = (bv @ Wo + bo).astype(f)

    def tr(b):  # [C] -> [P, NT] with b_t[p, t] = b[t*P + p]
        return np.ascontiguousarray(b.reshape(NT, P).T)

    def mask(CC, i0, width):
        pp = np.arange(P, dtype=np.int64)[:, None]
        gg = np.arange(width, dtype=np.int64)[None, :]
        return (pp <= gg - CC + i0).astype(bf)

    bias_t = np.ascontiguousarray(
        np.stack([tr(bq), tr(bk), tr(bo_eff),
                  np.ones((P, NT), f)], axis=1)
    )

    maps = []
    for core in range(8):
        b, p = core // 2, core % 2
        hi = 3 if p == 0 else 2
        # [P, chunk, ci, t'] = x[b, chunk*BLK+t', ci*P+p]
        xTv = np.ascontiguousarray(
            x[b].reshape(4, BLK, NT, P).transpose(3, 0, 2, 1)
        ).astype(bf)
        xkb = np.ascontiguousarray(xTv[:, [0, 1, hi]])
        xvb = np.ascontiguousarray(xTv[:, [2 * p, 2 * p + 1]])
        # queries: the core's four 256-row sub-blocks 2s+p, packed as two
        # 512-wide chunks [slot0|slot1], [slot2|slot3]
        xTq = np.ascontiguousarray(
            x[b].reshape(8, QW, NT, P).transpose(3, 0, 2, 1)
        ).astype(bf)
        xqb = np.ascontiguousarray(np.stack(
            [
                np.concatenate((xTq[:, p], xTq[:, p + 2]), axis=-1),
                np.concatenate((xTq[:, p + 4], xTq[:, p + 6]), axis=-1),
            ],
            axis=1,
        ))
        maps.append(
            {
                "xk": xkb,
                "xv": xvb,
                "xq": xqb,
                "Wk": Wk_t,
                "Wq": Wq_t,
                "Wv": Wv_t,
                "Wo": Wo_t,
                "bias_t": bias_t,
                "masks_t": mask(384 - 256 * p, 0, MW),
            }
        )
    return maps


def gather_output(results, T=T_, C=C_, B=B_):
    QW = T // 8
    out = np.empty((B, T, C), np.float32)
    for core in range(8):
        b, p = core // 2, core % 2
        oT = results[core]["outT"]
        for s in range(4):
            r0 = (2 * s + p) * QW
            out[b, r0:r0 + QW] = oT[:, s * QW:(s + 1) * QW].T
    return out


_NC_CACHE = {}


def kernel(x, Wq, bq, Wk, bk, Wv, bv, Wo, bo):
    from concourse.bass_utils import run_bass_kernel_spmd

    key = "full"
    if key not in _NC_CACHE:
        _NC_CACHE[key] = build_program()
    nc = _NC_CACHE[key]
    in_maps = make_core_inputs(x, Wq, bq, Wk, bk, Wv, bv, Wo, bo)
    res = run_bass_kernel_spmd(nc, in_maps, list(range(8))).results
    return gather_output(res)
